# revision 28
# baseline (speedup 1.0000x reference)
"""Trainium2 Bass kernel for GCNModelVAE (GCN encoder + inner-product decoder).

Math notes
----------
reference computes (eval mode, fp32):
    z1     = A @ (x @ W1) + b1          A = D^-1/2 (Adj + I) D^-1/2  (scatter form)
    mu     = A @ (z1 @ W2) + b2
    logvar = A @ (z1 @ W3) + b3
    adj    = sigmoid(mu @ mu.T)
    h1     = relu(adj[:,:,None] * Wc1.sum(0) + bc1)    # rank-1 in the scalar adj
    h2     = relu(h1 @ Wc2 + bc2)
    a_hat  = sigmoid(h2 @ Wc3 + bc3)

Host-side foldings (exact algebra, not approximations):
  * dense normalized adjacency A built from the edge list (index preprocessing),
    two-hop operator B = A @ A so
        mu     = B @ x @ (W1@W2) + rowsum(A) (x) (b1@W2) + b2
        logvar = B @ x @ (W1@W3) + rowsum(A) (x) (b1@W3) + b3
    (SGC-style operator folding; one AllGather of mu replaces a second
    propagation round).
  * decoder collapse: with bc1 == 0, bc2 == 0 and adj = sigmoid(..) > 0,
        relu(adj*w1s) = adj * relu(w1s)  elementwise, so
        a_hat[i,j,c] = sigmoid(adj_ij * u_c + bc3_c),
        u = relu(relu(Wc1.sum(0)) @ Wc2) @ Wc3          (4 floats)
    If bc1/bc2 are nonzero (never the case for this problem's inputs) a_hat
    falls back to an exact host computation; mu/logvar still come from the
    device.

Device layout (SPMD, 8 cores, nodes sharded 128 rows/core):
  per-core inputs: x [1024,256] (shared), BTb = B^T[:, blk] [1024,128],
                   W23 = [W1@W2 | W1@W3] [256,128], ident [128,128],
                   dconst=bc3 [1,4], jidx (gather indices for this core's
                   column window), mlcorr (rank-1 bias corr; program variant
                   only built when nonzero)
  r    = B_blk @ x                   (PE, 8 fp32 matmuls N=256, psum accum)
  rt   = r^T                         (2 PE transposes + 1 DVE copy)
  muT_blk = W12^T @ rt -> bf16       (2 fp32 matmuls + DVE cast)
  ml   = rt^T @ W23 (+ mlcorr)       (2 fp32 matmuls) = [mu|logvar] block
  zz_diag = muT_blk^T @ muT_blk, sigmoid+decode+store of the diagonal
           128x128 a_hat block — all while the AllGather is in flight
  AllGather(muT blk [64,128] bf16)   (16KB per rank, mesh, ~5us)
  indirect-DMA gathers pull this core's rotated 4-block muT window
  zz   = muT_blk.T @ muT_win         (1 bf16 matmul N=512)
  adj  = sigmoid(zz); out[:, 4j+c] = sigmoid(adj*u_c + bc3_c)  (ACT)
  store [128, 5*512] a_hat strip + fp32 mu/logvar blocks.

a_hat is symmetric (zz = mu @ mu.T), so each core computes only its own
block plus the next 4 (all pair distances 0..4); the host mirrors the
remaining blocks.  All big loads/stores are consolidated DMAs with
rearranged access patterns, split across the SP and ACT HWDGE queues; the
PE is kept HAM-warm with dummy matmuls during the load phase.
"""

import sys

for _p in ("/opt/trn_rl_repo", "/root/.axon_site/_ro/trn_rl_repo"):
    if _p not in sys.path:
        sys.path.append(_p)

import numpy as np

N = 1024
F = 256
H = 64
C = 4
NCORES = 8
BLK = N // NCORES  # 128
WJ = 5               # symmetric decoder: own block + next 4 (covers distances 0..4)
NJ = WJ * BLK        # 640 columns computed per core; the rest is mirrored on host

_PROGRAM_CACHE = {}
_EYE128 = np.eye(128, dtype=np.float32)


def _build_program(u, has_corr):
    import concourse.bass as bass
    from concourse import mybir

    AF = mybir.ActivationFunctionType
    f32 = mybir.dt.float32
    bf16 = mybir.dt.bfloat16

    nc = bass.Bass()

    x_in = nc.dram_tensor("x_in", [N, F], f32, kind="ExternalInput")
    BTb = nc.dram_tensor("BTb", [N, BLK], f32, kind="ExternalInput")
    W23 = nc.dram_tensor("W23", [F, 2 * H], f32, kind="ExternalInput")
    ident_in = nc.dram_tensor("ident_in", [128, 128], f32, kind="ExternalInput")
    dconst = nc.dram_tensor("dconst", [1, C], f32, kind="ExternalInput")
    mlcorr = nc.dram_tensor("mlcorr", [BLK, 2 * H], f32, kind="ExternalInput")
    jidx_in = nc.dram_tensor("jidx_in", [H, WJ - 1], mybir.dt.int32, kind="ExternalInput")

    ahat_o = nc.dram_tensor("ahat_o", [BLK, NJ * C], f32, kind="ExternalOutput")
    mu_o = nc.dram_tensor("mu_o", [BLK, H], f32, kind="ExternalOutput")
    lv_o = nc.dram_tensor("lv_o", [BLK, H], f32, kind="ExternalOutput")

    mublk_d = nc.dram_tensor("mublk_d", [H, BLK], bf16)
    muf_d = nc.dram_tensor("muf_d", [NCORES * H, BLK], bf16, addr_space="Shared")

    from contextlib import ExitStack

    with ExitStack() as ctx:
        def sb(name, shape, dt=f32):
            return ctx.enter_context(nc.sbuf_tensor(name, shape, dt))

        def ps(name):
            return ctx.enter_context(nc.psum_tensor(name, [128, 512], f32))

        def sem(name):
            return ctx.enter_context(nc.semaphore(name))

        xs = sb("xs", [128, NCORES * F])          # x[k*128+p, f] at [p, k*F+f]
        bt = sb("bt", [128, N])                   # BTb[k*128+p, m] at [p, k*128+m]
        w23s = sb("w23s", [128, 2 * 2 * H])       # W23[kh*128+p, c] at [p, kh*128+c]
        rs = sb("rs", [128, 2 * 128])
        rt = sb("rt", [128, 2 * 128])
        ml = sb("ml", [128, 2 * H])
        mlc = sb("mlc", [128, 2 * H])
        mutbb = sb("mutbb", [64, 128], bf16)      # bf16 muT own block
        mut = sb("mut", [64, NJ], bf16)           # bf16 muT, 5-block window
        adj = sb("adj", [128, NJ])
        outt = sb("outt", [128, NJ * C])
        jidx = sb("jidx", [64, WJ - 1], mybir.dt.int32)
        ident = sb("ident", [128, 128])
        bias3 = sb("bias3", [128, C])
        warm = sb("warm", [128, 8])
        dw = sb("dw", [128, 128])

        pr = ps("pr")
        ptq0, ptq1 = ps("ptq0"), ps("ptq1")
        pml = ps("pml")
        pmuT = ps("pmuT")
        pzz0, pzz1 = ps("pzz0"), ps("pzz1")

        names = ("s_ld", "s_ld2", "s_pr", "s_rs", "s_ptq", "s_rt", "s_pml",
                 "s_mlf", "s_pmuT", "s_mlb", "s_st1", "s_cc", "s_muf",
                 "s_zz", "s_adj", "s_dec", "s_sto", "s_wm", "s_jx",
                 "s_zzd", "s_adjd", "s_decd", "s_ldh")
        (s_ld, s_ld2, s_pr, s_rs, s_ptq, s_rt, s_pml,
         s_mlf, s_pmuT, s_mlb, s_st1, s_cc, s_mu5,
         s_zz, s_adj, s_dec, s_sto, s_wm, s_jx,
         s_zzd, s_adjd, s_decd, s_ldh) = [sem(n) for n in names]

        block = ctx.enter_context(nc.Block())
        outt_r = outt[:].rearrange("p (j c) -> p j c", c=C)

        def rearr_src(dram, steps):
            apx = dram[:]
            return bass.AP(tensor=apx.tensor, offset=apx.offset, ap=steps)

        @block.sync
        def _(sync):
            # x [1024,256] -> xs [128, 8*256], loaded in two halves
            sync.dma_start(
                xs[:, 0:4 * F], rearr_src(x_in, [[F, 128], [128 * F, 4], [1, F]])
            ).then_inc(s_ld, 16)
            xh1 = rearr_src(x_in, [[F, 128], [128 * F, 4], [1, F]])
            xh1.offset = 512 * F
            sync.dma_start(xs[:, 4 * F:8 * F], xh1).then_inc(s_ldh, 16)

            sync.wait_ge(s_mlf, 1)
            sync.dma_start(mu_o[:], ml[:, 0:H]).then_inc(s_sto, 16)
            sync.dma_start(lv_o[:], ml[:, H:2 * H]).then_inc(s_sto, 16)

            sync.wait_ge(s_decd, 4)
            sync.dma_start(ahat_o[:, 0:512], outt[:, 0:512]).then_inc(s_sto, 16)
            sync.wait_ge(s_dec, 4)
            sync.dma_start(ahat_o[:, 512:1536], outt[:, 512:1536]).then_inc(s_sto, 16)
            sync.wait_ge(s_sto, 80)

        @block.scalar
        def _(scalar):
            # BTb [1024,128] -> bt [128, 8*128], two halves (parallel to x load)
            scalar.dma_start(
                bt[:, 0:512], rearr_src(BTb, [[BLK, 128], [128 * BLK, 4], [1, BLK]])
            ).then_inc(s_ld, 16)
            bh1 = rearr_src(BTb, [[BLK, 128], [128 * BLK, 4], [1, BLK]])
            bh1.offset = 512 * BLK
            scalar.dma_start(bt[:, 512:1024], bh1).then_inc(s_ldh, 16)
            scalar.dma_start(ident[:], ident_in[:]).then_inc(s_ld2, 16)
            # W23 [256,128] -> w23s [128, 2*128]
            scalar.dma_start(
                w23s[:], rearr_src(W23, [[2 * H, 128], [128 * 2 * H, 2], [1, 2 * H]])
            ).then_inc(s_ld2, 16)
            dc = dconst[:]
            dc_b = bass.AP(tensor=dc.tensor, offset=dc.offset, ap=[[0, 128], dc.ap[-1]])
            scalar.dma_start(bias3[:], dc_b).then_inc(s_ld2, 16)
            if has_corr:
                scalar.dma_start(mlc[:], mlcorr[:]).then_inc(s_ld2, 16)
            # warm the sigmoid ACT table while DMAs/matmuls run
            scalar.wait_ge(s_ld2, 64 if has_corr else 48)
            nc.scalar.activation(warm[:], ident[:, 0:8], AF.Sigmoid)

            # diagonal block: sigmoid+decode while the AllGather is in flight
            scalar.wait_ge(s_zzd, 1)
            nc.scalar.activation(adj[:, 0:128], ptq0[:, 0:128], AF.Sigmoid).then_inc(s_adjd, 1)
            scalar.wait_ge(s_adjd, 1)
            for c in range(C):
                nc.scalar.activation(
                    outt_r[:, 0:128, c],
                    adj[:, 0:128],
                    AF.Sigmoid,
                    bias=bias3[:, c:c + 1],
                    scale=float(u[c]),
                ).then_inc(s_decd, 1)
            # off-diagonal blocks after the gather
            scalar.wait_ge(s_zz, 1)
            nc.scalar.activation(adj[:, 128:NJ], pzz0[:, 0:512], AF.Sigmoid).then_inc(s_adj, 1)
            scalar.wait_ge(s_adj, 1)
            for c in range(C):
                nc.scalar.activation(
                    outt_r[:, 128:NJ, c],
                    adj[:, 128:NJ],
                    AF.Sigmoid,
                    bias=bias3[:, c:c + 1],
                    scale=float(u[c]),
                ).then_inc(s_dec, 1)
            # store the tail from the ACT HWDGE queue; self-wait ensures the
            # decode writes have retired before the DMA reads SBUF
            scalar.wait_ge(s_dec, 4)
            scalar.dma_start(ahat_o[:, 1536:NJ * C], outt[:, 1536:NJ * C]).then_inc(s_sto, 16)

        @block.gpsimd
        def _(gpsimd):
            gpsimd.dma_start(jidx[:], jidx_in[0:64, :]).then_inc(s_jx, 16)
            gpsimd.wait_ge(s_mlb, 1)
            gpsimd.dma_start(mublk_d[:], mutbb[0:64, :]).then_inc(s_st1, 16)
            gpsimd.wait_ge(s_st1, 16)
            gpsimd.collective_compute(
                "AllGather",
                mybir.AluOpType.bypass,
                ins=[mublk_d[:]],
                outs=[muf_d[:]],
                replica_groups=[list(range(NCORES))],
            ).then_inc(s_cc, 1)
            gpsimd.wait_ge(s_cc, 1)
            gpsimd.wait_ge(s_jx, 16)
            for b in range(1, WJ):
                gpsimd.indirect_dma_start(
                    out=mut[0:64, b * 128:(b + 1) * 128],
                    out_offset=None,
                    in_=muf_d[:],
                    in_offset=bass.IndirectOffsetOnAxis(ap=jidx[:, b - 1:b], axis=0),
                ).then_inc(s_mu5, 16)

        @block.tensor
        def _(tensor):
            # spin the PE so HAM un-throttles before the real matmuls arrive
            tensor.wait_ge(s_wm, 1)
            for w in range(8):
                nc.tensor.matmul(pzz0[:, 0:128], lhsT=dw[:], rhs=dw[:],
                                 start=True, stop=True)
            tensor.wait_ge(s_ld, 32)
            for k in range(NCORES):
                if k == 4:
                    tensor.wait_ge(s_ldh, 32)
                mm = nc.tensor.matmul(
                    pr[:, 0:F], lhsT=bt[:, k * 128:(k + 1) * 128],
                    rhs=xs[:, k * F:(k + 1) * F],
                    start=(k == 0), stop=(k == NCORES - 1),
                )
            mm.then_inc(s_pr, 1)
            tensor.wait_ge(s_rs, 1)
            tensor.wait_ge(s_ld2, 64 if has_corr else 48)
            nc.tensor.transpose(ptq0[:, 0:128], rs[:, 0:128], ident[:])
            nc.tensor.transpose(ptq0[:, 128:256], rs[:, 128:256], ident[:]).then_inc(s_ptq, 1)
            tensor.wait_ge(s_rt, 1)
            # muT = W12^T @ r^T directly from rt (same sum order as ml's mu)
            nc.tensor.matmul(pmuT[0:64, 0:128], lhsT=w23s[:, 0:H], rhs=rt[:, 0:128],
                             start=True, stop=False)
            nc.tensor.matmul(pmuT[0:64, 0:128], lhsT=w23s[:, 128:128 + H], rhs=rt[:, 128:256],
                             start=False, stop=True).then_inc(s_pmuT, 1)
            nc.tensor.matmul(pml[:, 0:2 * H], lhsT=rt[:, 0:128], rhs=w23s[:, 0:128],
                             start=True, stop=False)
            nc.tensor.matmul(pml[:, 0:2 * H], lhsT=rt[:, 128:256], rhs=w23s[:, 128:256],
                             start=False, stop=True).then_inc(s_pml, 1)
            # diagonal zz block from our own mu while the AllGather runs
            tensor.wait_ge(s_mlb, 1)
            nc.tensor.matmul(ptq0[:, 0:128], lhsT=mutbb[0:64, :], rhs=mutbb[0:64, :],
                             start=True, stop=True).then_inc(s_zzd, 1)
            # keep HAM warm through the (~5us on HW) collective: this burst is
            # gated on the AllGather STARTING, so it overlaps the collective
            # and the gathers rather than delaying them
            tensor.wait_ge(s_st1, 16)
            for w in range(7):
                nc.tensor.matmul(pr[:, 0:128], lhsT=dw[:], rhs=dw[:],
                                 start=True, stop=True)
            tensor.wait_ge(s_mu5, 16 * (WJ - 1))
            nc.tensor.matmul(pzz0[:, 0:512], lhsT=mutbb[0:64, :], rhs=mut[0:64, 128:NJ],
                             start=True, stop=True).then_inc(s_zz, 1)

        @block.vector
        def _(vector):
            nc.vector.memset(dw[:], 0.0).then_inc(s_wm, 1)
            vector.wait_ge(s_pr, 1)
            nc.vector.tensor_copy(rs[:], pr[:, 0:F]).then_inc(s_rs, 1)
            vector.wait_ge(s_ptq, 1)
            nc.vector.tensor_copy(rt[:], ptq0[:, 0:256]).then_inc(s_rt, 1)
            vector.wait_ge(s_pmuT, 1)
            nc.vector.tensor_copy(mutbb[0:64, :], pmuT[0:64, 0:128]).then_inc(s_mlb, 1)
            vector.wait_ge(s_pml, 1)
            if has_corr:
                vector.wait_ge(s_ld2, 64)
                nc.vector.tensor_add(ml[:], pml[:, 0:2 * H], mlc[:]).then_inc(s_mlf, 1)
            else:
                nc.vector.tensor_copy(ml[:], pml[:, 0:2 * H]).then_inc(s_mlf, 1)

    return nc


def _get_program(u, has_corr):
    """Programs are cached per decode-scale vector u (baked as immediates)."""
    key = (tuple(np.asarray(u, np.float32).tolist()), bool(has_corr))
    if key not in _PROGRAM_CACHE:
        _PROGRAM_CACHE[key] = _build_program(np.asarray(u, np.float32), bool(has_corr))
    return _PROGRAM_CACHE[key]


def _host_prep(inputs):
    x = np.asarray(inputs["x"], np.float32)
    src = np.asarray(inputs["src"]).astype(np.int64)
    dst = np.asarray(inputs["dst"]).astype(np.int64)
    W1 = np.asarray(inputs["W1"], np.float32)
    b1 = np.asarray(inputs["b1"], np.float32)
    W2 = np.asarray(inputs["W2"], np.float32)
    b2 = np.asarray(inputs["b2"], np.float32)
    W3 = np.asarray(inputs["W3"], np.float32)
    b3 = np.asarray(inputs["b3"], np.float32)
    Wc1 = np.asarray(inputs["Wc1"], np.float32)
    bc1 = np.asarray(inputs["bc1"], np.float32)
    Wc2 = np.asarray(inputs["Wc2"], np.float32)
    bc2 = np.asarray(inputs["bc2"], np.float32)
    Wc3 = np.asarray(inputs["Wc3"], np.float32)
    bc3 = np.asarray(inputs["bc3"], np.float32)

    n = x.shape[0]
    assert n == N and x.shape[1] == F

    # graph operator prep (index preprocessing + operator folding)
    deg = np.ones(n, np.float32)
    np.add.at(deg, dst, 1.0)
    dis = (1.0 / np.sqrt(deg)).astype(np.float32)
    AT = np.zeros((n, n), np.float32)
    np.add.at(AT, (src, dst), dis[src] * dis[dst])
    di = np.arange(n)
    AT[di, di] += dis * dis
    BT = AT @ AT  # (A@A)^T
    rowsumA = AT.sum(axis=0)  # A @ ones

    W23 = np.concatenate([W1 @ W2, W1 @ W3], axis=1).astype(np.float32)

    # rank-1 encoder bias corrections (zero for this problem's inputs)
    mlcorr_full = np.concatenate(
        [rowsumA[:, None] * (b1 @ W2)[None, :] + b2[None, :],
         rowsumA[:, None] * (b1 @ W3)[None, :] + b3[None, :]],
        axis=1).astype(np.float32)

    has_corr = bool(np.any(mlcorr_full != 0.0))
    w1s = Wc1.sum(axis=0)
    fast = bool(np.all(bc1 == 0.0) and np.all(bc2 == 0.0))
    if fast:
        u = (np.maximum(np.maximum(w1s, 0.0) @ Wc2, 0.0) @ Wc3).astype(np.float32)
    else:
        u = np.zeros(C, np.float32)

    in_maps = []
    q = np.arange(H, dtype=np.int32)
    for i in range(NCORES):
        jidx = np.stack(
            [((i + b) % NCORES) * H + q for b in range(1, WJ)], axis=1
        ).astype(np.int32)
        in_maps.append(
            dict(
                x_in=x,
                BTb=np.ascontiguousarray(BT[:, i * BLK:(i + 1) * BLK]),
                W23=W23,
                ident_in=_EYE128,
                dconst=bc3.reshape(1, C),
                mlcorr=mlcorr_full[i * BLK:(i + 1) * BLK, :],
                jidx_in=jidx,
            )
        )
    dec = dict(fast=fast, w1s=w1s, Wc2=Wc2, bc2=bc2, Wc3=Wc3, bc3=bc3, bc1=bc1,
               has_corr=has_corr)
    return u, in_maps, dec


def kernel(**inputs):
    u, in_maps, dec = _host_prep(inputs)
    nc = _get_program(u, dec["has_corr"])

    from concourse.bass_utils import run_bass_kernel_spmd

    res = run_bass_kernel_spmd(nc, in_maps, list(range(NCORES)))

    mu = np.concatenate([res.results[i]["mu_o"] for i in range(NCORES)], axis=0)
    logvar = np.concatenate([res.results[i]["lv_o"] for i in range(NCORES)], axis=0)
    a_hat = np.empty((N, N, C), np.float32)
    for i in range(NCORES):
        strip = res.results[i]["ahat_o"].reshape(BLK, WJ, BLK, C)
        for b in range(WJ):
            jb = (i + b) % NCORES
            a_hat[i * BLK:(i + 1) * BLK, jb * BLK:(jb + 1) * BLK, :] = strip[:, b]
    # a_hat is symmetric (zz = mu @ mu.T); mirror the uncomputed blocks
    for ib in range(NCORES):
        for jb in range(NCORES):
            if (jb - ib) % NCORES > WJ - 1:
                a_hat[ib * BLK:(ib + 1) * BLK, jb * BLK:(jb + 1) * BLK, :] = (
                    a_hat[jb * BLK:(jb + 1) * BLK, ib * BLK:(ib + 1) * BLK, :]
                    .transpose(1, 0, 2)
                )

    if not dec["fast"]:
        # exact host fallback (never hit for this problem's inputs)
        adj = 1.0 / (1.0 + np.exp(-(mu @ mu.T)))
        w1s, Wc2, bc2, Wc3, bc3, bc1 = (dec["w1s"], dec["Wc2"], dec["bc2"],
                                        dec["Wc3"], dec["bc3"], dec["bc1"])
        a_hat = np.empty((N, N, C), np.float32)
        for i0 in range(0, N, 64):
            h = np.maximum(adj[i0:i0 + 64][:, :, None] * w1s[None, None, :] + bc1, 0.0)
            h = np.maximum(h @ Wc2 + bc2, 0.0)
            a_hat[i0:i0 + 64] = 1.0 / (1.0 + np.exp(-(h @ Wc3 + bc3)))

    return a_hat, mu, logvar


# revision 29
# speedup vs baseline: 1.0057x; 1.0057x over previous
"""Trainium2 Bass kernel for GCNModelVAE (GCN encoder + inner-product decoder).

Math notes
----------
reference computes (eval mode, fp32):
    z1     = A @ (x @ W1) + b1          A = D^-1/2 (Adj + I) D^-1/2  (scatter form)
    mu     = A @ (z1 @ W2) + b2
    logvar = A @ (z1 @ W3) + b3
    adj    = sigmoid(mu @ mu.T)
    h1     = relu(adj[:,:,None] * Wc1.sum(0) + bc1)    # rank-1 in the scalar adj
    h2     = relu(h1 @ Wc2 + bc2)
    a_hat  = sigmoid(h2 @ Wc3 + bc3)

Host-side foldings (exact algebra, not approximations):
  * dense normalized adjacency A built from the edge list (index preprocessing),
    two-hop operator B = A @ A so
        mu     = B @ x @ (W1@W2) + rowsum(A) (x) (b1@W2) + b2
        logvar = B @ x @ (W1@W3) + rowsum(A) (x) (b1@W3) + b3
    (SGC-style operator folding; one AllGather of mu replaces a second
    propagation round).
  * decoder collapse: with bc1 == 0, bc2 == 0 and adj = sigmoid(..) > 0,
        relu(adj*w1s) = adj * relu(w1s)  elementwise, so
        a_hat[i,j,c] = sigmoid(adj_ij * u_c + bc3_c),
        u = relu(relu(Wc1.sum(0)) @ Wc2) @ Wc3          (4 floats)
    If bc1/bc2 are nonzero (never the case for this problem's inputs) a_hat
    falls back to an exact host computation; mu/logvar still come from the
    device.

Device layout (SPMD, 8 cores, nodes sharded 128 rows/core):
  per-core inputs: x [1024,256] (shared), BTb = B^T[:, blk] [1024,128],
                   W23 = [W1@W2 | W1@W3] [256,128], ident [128,128],
                   dconst=bc3 [1,4], jidx (gather indices for this core's
                   column window), mlcorr (rank-1 bias corr; program variant
                   only built when nonzero)
  r    = B_blk @ x                   (PE, 8 fp32 matmuls N=256, psum accum)
  rt   = r^T                         (2 PE transposes + 1 DVE copy)
  muT_blk = W12^T @ rt -> bf16       (2 fp32 matmuls + DVE cast)
  ml   = rt^T @ W23 (+ mlcorr)       (2 fp32 matmuls) = [mu|logvar] block
  zz_diag = muT_blk^T @ muT_blk, sigmoid+decode+store of the diagonal
           128x128 a_hat block — all while the AllGather is in flight
  AllGather(muT blk [64,128] bf16)   (16KB per rank, mesh, ~5us)
  indirect-DMA gathers pull this core's rotated 4-block muT window
  zz   = muT_blk.T @ muT_win         (1 bf16 matmul N=512)
  adj  = sigmoid(zz); out[:, 4j+c] = sigmoid(adj*u_c + bc3_c)  (ACT)
  store [128, 5*512] a_hat strip + fp32 mu/logvar blocks.

a_hat is symmetric (zz = mu @ mu.T), so each core computes only its own
block plus the next 4 (all pair distances 0..4); the host mirrors the
remaining blocks.  All big loads/stores are consolidated DMAs with
rearranged access patterns, split across the SP and ACT HWDGE queues; the
PE is kept HAM-warm with dummy matmuls during the load phase.
"""

import sys

for _p in ("/opt/trn_rl_repo", "/root/.axon_site/_ro/trn_rl_repo"):
    if _p not in sys.path:
        sys.path.append(_p)

import numpy as np

N = 1024
F = 256
H = 64
C = 4
NCORES = 8
BLK = N // NCORES  # 128
WJ = 5               # symmetric decoder: own block + next 4 (covers distances 0..4)
NJ = WJ * BLK        # 640 columns computed per core; the rest is mirrored on host

_PROGRAM_CACHE = {}
_EYE128 = np.eye(128, dtype=np.float32)


def _build_program(u, has_corr):
    import concourse.bass as bass
    from concourse import mybir

    AF = mybir.ActivationFunctionType
    f32 = mybir.dt.float32
    bf16 = mybir.dt.bfloat16

    nc = bass.Bass()

    x_in = nc.dram_tensor("x_in", [N, F], f32, kind="ExternalInput")
    BTb = nc.dram_tensor("BTb", [N, BLK], f32, kind="ExternalInput")
    W23 = nc.dram_tensor("W23", [F, 2 * H], f32, kind="ExternalInput")
    ident_in = nc.dram_tensor("ident_in", [128, 128], f32, kind="ExternalInput")
    dconst = nc.dram_tensor("dconst", [1, C], f32, kind="ExternalInput")
    mlcorr = nc.dram_tensor("mlcorr", [BLK, 2 * H], f32, kind="ExternalInput")
    jidx_in = nc.dram_tensor("jidx_in", [H, WJ - 1], mybir.dt.int32, kind="ExternalInput")

    ahat_o = nc.dram_tensor("ahat_o", [BLK, NJ * C], f32, kind="ExternalOutput")
    mu_o = nc.dram_tensor("mu_o", [BLK, H], f32, kind="ExternalOutput")
    lv_o = nc.dram_tensor("lv_o", [BLK, H], f32, kind="ExternalOutput")

    mublk_d = nc.dram_tensor("mublk_d", [H, BLK], bf16)
    muf_d = nc.dram_tensor("muf_d", [NCORES * H, BLK], bf16, addr_space="Shared")

    from contextlib import ExitStack

    with ExitStack() as ctx:
        def sb(name, shape, dt=f32):
            return ctx.enter_context(nc.sbuf_tensor(name, shape, dt))

        def ps(name):
            return ctx.enter_context(nc.psum_tensor(name, [128, 512], f32))

        def sem(name):
            return ctx.enter_context(nc.semaphore(name))

        xs = sb("xs", [128, NCORES * F])          # x[k*128+p, f] at [p, k*F+f]
        bt = sb("bt", [128, N])                   # BTb[k*128+p, m] at [p, k*128+m]
        w23s = sb("w23s", [128, 2 * 2 * H])       # W23[kh*128+p, c] at [p, kh*128+c]
        rs = sb("rs", [128, 2 * 128])
        rt = sb("rt", [128, 2 * 128])
        ml = sb("ml", [128, 2 * H])
        mlc = sb("mlc", [128, 2 * H])
        mutbb = sb("mutbb", [64, 128], bf16)      # bf16 muT own block
        mut = sb("mut", [64, NJ], bf16)           # bf16 muT, 5-block window
        adj = sb("adj", [128, NJ])
        outt = sb("outt", [128, NJ * C])
        jidx = sb("jidx", [64, WJ - 1], mybir.dt.int32)
        ident = sb("ident", [128, 128])
        bias3 = sb("bias3", [128, C])
        warm = sb("warm", [128, 8])
        dw = sb("dw", [128, 128])

        pr = ps("pr")
        ptq0, ptq1 = ps("ptq0"), ps("ptq1")
        pml = ps("pml")
        pmuT = ps("pmuT")
        pzz0, pzz1 = ps("pzz0"), ps("pzz1")

        names = ("s_ld", "s_ld2", "s_pr", "s_rs", "s_ptq", "s_rt", "s_pml",
                 "s_mlf", "s_pmuT", "s_mlb", "s_st1", "s_cc", "s_muf",
                 "s_zz", "s_adj", "s_dec", "s_sto", "s_wm", "s_jx",
                 "s_zzd", "s_adjd", "s_decd", "s_ldh")
        (s_ld, s_ld2, s_pr, s_rs, s_ptq, s_rt, s_pml,
         s_mlf, s_pmuT, s_mlb, s_st1, s_cc, s_mu5,
         s_zz, s_adj, s_dec, s_sto, s_wm, s_jx,
         s_zzd, s_adjd, s_decd, s_ldh) = [sem(n) for n in names]

        block = ctx.enter_context(nc.Block())
        outt_r = outt[:].rearrange("p (j c) -> p j c", c=C)

        def rearr_src(dram, steps):
            apx = dram[:]
            return bass.AP(tensor=apx.tensor, offset=apx.offset, ap=steps)

        @block.sync
        def _(sync):
            # x [1024,256] -> xs [128, 8*256], loaded in two halves
            sync.dma_start(
                xs[:, 0:4 * F], rearr_src(x_in, [[F, 128], [128 * F, 4], [1, F]])
            ).then_inc(s_ld, 16)
            xh1 = rearr_src(x_in, [[F, 128], [128 * F, 4], [1, F]])
            xh1.offset = 512 * F
            sync.dma_start(xs[:, 4 * F:8 * F], xh1).then_inc(s_ldh, 16)

            sync.wait_ge(s_mlf, 1)
            sync.dma_start(mu_o[:], ml[:, 0:H]).then_inc(s_sto, 16)
            sync.dma_start(lv_o[:], ml[:, H:2 * H]).then_inc(s_sto, 16)

            sync.wait_ge(s_decd, 4)
            sync.dma_start(ahat_o[:, 0:512], outt[:, 0:512]).then_inc(s_sto, 16)
            sync.wait_ge(s_dec, 4)
            sync.dma_start(ahat_o[:, 512:1536], outt[:, 512:1536]).then_inc(s_sto, 16)
            sync.wait_ge(s_sto, 80)

        @block.scalar
        def _(scalar):
            # BTb [1024,128] -> bt [128, 8*128], two halves (parallel to x load)
            scalar.dma_start(
                bt[:, 0:512], rearr_src(BTb, [[BLK, 128], [128 * BLK, 4], [1, BLK]])
            ).then_inc(s_ld, 16)
            bh1 = rearr_src(BTb, [[BLK, 128], [128 * BLK, 4], [1, BLK]])
            bh1.offset = 512 * BLK
            scalar.dma_start(bt[:, 512:1024], bh1).then_inc(s_ldh, 16)
            scalar.dma_start(ident[:], ident_in[:]).then_inc(s_ld2, 16)
            # W23 [256,128] -> w23s [128, 2*128]
            scalar.dma_start(
                w23s[:], rearr_src(W23, [[2 * H, 128], [128 * 2 * H, 2], [1, 2 * H]])
            ).then_inc(s_ld2, 16)
            dc = dconst[:]
            dc_b = bass.AP(tensor=dc.tensor, offset=dc.offset, ap=[[0, 128], dc.ap[-1]])
            scalar.dma_start(bias3[:], dc_b).then_inc(s_ld2, 16)
            if has_corr:
                scalar.dma_start(mlc[:], mlcorr[:]).then_inc(s_ld2, 16)
            # warm the sigmoid ACT table while DMAs/matmuls run
            scalar.wait_ge(s_ld2, 64 if has_corr else 48)
            nc.scalar.activation(warm[:], ident[:, 0:8], AF.Sigmoid)

            # diagonal block: sigmoid+decode while the AllGather is in flight
            scalar.wait_ge(s_zzd, 1)
            nc.scalar.activation(adj[:, 0:128], ptq0[:, 0:128], AF.Sigmoid).then_inc(s_adjd, 1)
            scalar.wait_ge(s_adjd, 1)
            for c in range(C):
                nc.scalar.activation(
                    outt_r[:, 0:128, c],
                    adj[:, 0:128],
                    AF.Sigmoid,
                    bias=bias3[:, c:c + 1],
                    scale=float(u[c]),
                ).then_inc(s_decd, 1)
            # off-diagonal blocks after the gather
            scalar.wait_ge(s_zz, 1)
            nc.scalar.activation(adj[:, 128:NJ], pzz0[:, 0:512], AF.Sigmoid).then_inc(s_adj, 1)
            scalar.wait_ge(s_adj, 1)
            for c in range(C):
                nc.scalar.activation(
                    outt_r[:, 128:NJ, c],
                    adj[:, 128:NJ],
                    AF.Sigmoid,
                    bias=bias3[:, c:c + 1],
                    scale=float(u[c]),
                ).then_inc(s_dec, 1)
            # store the tail from the ACT HWDGE queue; self-wait ensures the
            # decode writes have retired before the DMA reads SBUF
            scalar.wait_ge(s_dec, 4)
            scalar.dma_start(ahat_o[:, 1536:NJ * C], outt[:, 1536:NJ * C]).then_inc(s_sto, 16)

        @block.gpsimd
        def _(gpsimd):
            gpsimd.dma_start(jidx[:], jidx_in[0:64, :]).then_inc(s_jx, 16)
            gpsimd.wait_ge(s_mlb, 1)
            gpsimd.dma_start(mublk_d[:], mutbb[0:64, :]).then_inc(s_st1, 16)
            gpsimd.wait_ge(s_st1, 16)
            gpsimd.collective_compute(
                "AllGather",
                mybir.AluOpType.bypass,
                ins=[mublk_d[:]],
                outs=[muf_d[:]],
                replica_groups=[list(range(NCORES))],
            ).then_inc(s_cc, 1)
            gpsimd.wait_ge(s_cc, 1)
            gpsimd.wait_ge(s_jx, 16)
            for b in range(1, WJ):
                gpsimd.indirect_dma_start(
                    out=mut[0:64, b * 128:(b + 1) * 128],
                    out_offset=None,
                    in_=muf_d[:],
                    in_offset=bass.IndirectOffsetOnAxis(ap=jidx[:, b - 1:b], axis=0),
                ).then_inc(s_mu5, 16)

        @block.tensor
        def _(tensor):
            # spin the PE so HAM un-throttles before the real matmuls arrive
            tensor.wait_ge(s_wm, 1)
            for w in range(7):
                nc.tensor.matmul(pzz0[:, 0:128], lhsT=dw[:], rhs=dw[:],
                                 start=True, stop=True)
            tensor.wait_ge(s_ld, 32)
            for k in range(NCORES):
                if k == 4:
                    tensor.wait_ge(s_ldh, 32)
                mm = nc.tensor.matmul(
                    pr[:, 0:F], lhsT=bt[:, k * 128:(k + 1) * 128],
                    rhs=xs[:, k * F:(k + 1) * F],
                    start=(k == 0), stop=(k == NCORES - 1),
                )
            mm.then_inc(s_pr, 1)
            tensor.wait_ge(s_rs, 1)
            tensor.wait_ge(s_ld2, 64 if has_corr else 48)
            nc.tensor.transpose(ptq0[:, 0:128], rs[:, 0:128], ident[:])
            nc.tensor.transpose(ptq0[:, 128:256], rs[:, 128:256], ident[:]).then_inc(s_ptq, 1)
            tensor.wait_ge(s_rt, 1)
            # muT = W12^T @ r^T directly from rt (same sum order as ml's mu)
            nc.tensor.matmul(pmuT[0:64, 0:128], lhsT=w23s[:, 0:H], rhs=rt[:, 0:128],
                             start=True, stop=False)
            nc.tensor.matmul(pmuT[0:64, 0:128], lhsT=w23s[:, 128:128 + H], rhs=rt[:, 128:256],
                             start=False, stop=True).then_inc(s_pmuT, 1)
            nc.tensor.matmul(pml[:, 0:2 * H], lhsT=rt[:, 0:128], rhs=w23s[:, 0:128],
                             start=True, stop=False)
            nc.tensor.matmul(pml[:, 0:2 * H], lhsT=rt[:, 128:256], rhs=w23s[:, 128:256],
                             start=False, stop=True).then_inc(s_pml, 1)
            # diagonal zz block from our own mu while the AllGather runs
            tensor.wait_ge(s_mlb, 1)
            nc.tensor.matmul(ptq0[:, 0:128], lhsT=mutbb[0:64, :], rhs=mutbb[0:64, :],
                             start=True, stop=True).then_inc(s_zzd, 1)
            # keep HAM warm through the (~5us on HW) collective: this burst is
            # gated on the AllGather STARTING, so it overlaps the collective
            # and the gathers rather than delaying them
            tensor.wait_ge(s_st1, 16)
            for w in range(7):
                nc.tensor.matmul(pr[:, 0:128], lhsT=dw[:], rhs=dw[:],
                                 start=True, stop=True)
            tensor.wait_ge(s_mu5, 16 * (WJ - 1))
            nc.tensor.matmul(pzz0[:, 0:512], lhsT=mutbb[0:64, :], rhs=mut[0:64, 128:NJ],
                             start=True, stop=True).then_inc(s_zz, 1)

        @block.vector
        def _(vector):
            nc.vector.memset(dw[:], 0.0).then_inc(s_wm, 1)
            vector.wait_ge(s_pr, 1)
            nc.vector.tensor_copy(rs[:], pr[:, 0:F]).then_inc(s_rs, 1)
            vector.wait_ge(s_ptq, 1)
            nc.vector.tensor_copy(rt[:], ptq0[:, 0:256]).then_inc(s_rt, 1)
            vector.wait_ge(s_pmuT, 1)
            nc.vector.tensor_copy(mutbb[0:64, :], pmuT[0:64, 0:128]).then_inc(s_mlb, 1)
            vector.wait_ge(s_pml, 1)
            if has_corr:
                vector.wait_ge(s_ld2, 64)
                nc.vector.tensor_add(ml[:], pml[:, 0:2 * H], mlc[:]).then_inc(s_mlf, 1)
            else:
                nc.vector.tensor_copy(ml[:], pml[:, 0:2 * H]).then_inc(s_mlf, 1)

    return nc


def _get_program(u, has_corr):
    """Programs are cached per decode-scale vector u (baked as immediates)."""
    key = (tuple(np.asarray(u, np.float32).tolist()), bool(has_corr))
    if key not in _PROGRAM_CACHE:
        _PROGRAM_CACHE[key] = _build_program(np.asarray(u, np.float32), bool(has_corr))
    return _PROGRAM_CACHE[key]


def _host_prep(inputs):
    x = np.asarray(inputs["x"], np.float32)
    src = np.asarray(inputs["src"]).astype(np.int64)
    dst = np.asarray(inputs["dst"]).astype(np.int64)
    W1 = np.asarray(inputs["W1"], np.float32)
    b1 = np.asarray(inputs["b1"], np.float32)
    W2 = np.asarray(inputs["W2"], np.float32)
    b2 = np.asarray(inputs["b2"], np.float32)
    W3 = np.asarray(inputs["W3"], np.float32)
    b3 = np.asarray(inputs["b3"], np.float32)
    Wc1 = np.asarray(inputs["Wc1"], np.float32)
    bc1 = np.asarray(inputs["bc1"], np.float32)
    Wc2 = np.asarray(inputs["Wc2"], np.float32)
    bc2 = np.asarray(inputs["bc2"], np.float32)
    Wc3 = np.asarray(inputs["Wc3"], np.float32)
    bc3 = np.asarray(inputs["bc3"], np.float32)

    n = x.shape[0]
    assert n == N and x.shape[1] == F

    # graph operator prep (index preprocessing + operator folding)
    deg = np.ones(n, np.float32)
    np.add.at(deg, dst, 1.0)
    dis = (1.0 / np.sqrt(deg)).astype(np.float32)
    AT = np.zeros((n, n), np.float32)
    np.add.at(AT, (src, dst), dis[src] * dis[dst])
    di = np.arange(n)
    AT[di, di] += dis * dis
    BT = AT @ AT  # (A@A)^T
    rowsumA = AT.sum(axis=0)  # A @ ones

    W23 = np.concatenate([W1 @ W2, W1 @ W3], axis=1).astype(np.float32)

    # rank-1 encoder bias corrections (zero for this problem's inputs)
    mlcorr_full = np.concatenate(
        [rowsumA[:, None] * (b1 @ W2)[None, :] + b2[None, :],
         rowsumA[:, None] * (b1 @ W3)[None, :] + b3[None, :]],
        axis=1).astype(np.float32)

    has_corr = bool(np.any(mlcorr_full != 0.0))
    w1s = Wc1.sum(axis=0)
    fast = bool(np.all(bc1 == 0.0) and np.all(bc2 == 0.0))
    if fast:
        u = (np.maximum(np.maximum(w1s, 0.0) @ Wc2, 0.0) @ Wc3).astype(np.float32)
    else:
        u = np.zeros(C, np.float32)

    in_maps = []
    q = np.arange(H, dtype=np.int32)
    for i in range(NCORES):
        jidx = np.stack(
            [((i + b) % NCORES) * H + q for b in range(1, WJ)], axis=1
        ).astype(np.int32)
        in_maps.append(
            dict(
                x_in=x,
                BTb=np.ascontiguousarray(BT[:, i * BLK:(i + 1) * BLK]),
                W23=W23,
                ident_in=_EYE128,
                dconst=bc3.reshape(1, C),
                mlcorr=mlcorr_full[i * BLK:(i + 1) * BLK, :],
                jidx_in=jidx,
            )
        )
    dec = dict(fast=fast, w1s=w1s, Wc2=Wc2, bc2=bc2, Wc3=Wc3, bc3=bc3, bc1=bc1,
               has_corr=has_corr)
    return u, in_maps, dec


def kernel(**inputs):
    u, in_maps, dec = _host_prep(inputs)
    nc = _get_program(u, dec["has_corr"])

    from concourse.bass_utils import run_bass_kernel_spmd

    res = run_bass_kernel_spmd(nc, in_maps, list(range(NCORES)))

    mu = np.concatenate([res.results[i]["mu_o"] for i in range(NCORES)], axis=0)
    logvar = np.concatenate([res.results[i]["lv_o"] for i in range(NCORES)], axis=0)
    a_hat = np.empty((N, N, C), np.float32)
    for i in range(NCORES):
        strip = res.results[i]["ahat_o"].reshape(BLK, WJ, BLK, C)
        for b in range(WJ):
            jb = (i + b) % NCORES
            a_hat[i * BLK:(i + 1) * BLK, jb * BLK:(jb + 1) * BLK, :] = strip[:, b]
    # a_hat is symmetric (zz = mu @ mu.T); mirror the uncomputed blocks
    for ib in range(NCORES):
        for jb in range(NCORES):
            if (jb - ib) % NCORES > WJ - 1:
                a_hat[ib * BLK:(ib + 1) * BLK, jb * BLK:(jb + 1) * BLK, :] = (
                    a_hat[jb * BLK:(jb + 1) * BLK, ib * BLK:(ib + 1) * BLK, :]
                    .transpose(1, 0, 2)
                )

    if not dec["fast"]:
        # exact host fallback (never hit for this problem's inputs)
        adj = 1.0 / (1.0 + np.exp(-(mu @ mu.T)))
        w1s, Wc2, bc2, Wc3, bc3, bc1 = (dec["w1s"], dec["Wc2"], dec["bc2"],
                                        dec["Wc3"], dec["bc3"], dec["bc1"])
        a_hat = np.empty((N, N, C), np.float32)
        for i0 in range(0, N, 64):
            h = np.maximum(adj[i0:i0 + 64][:, :, None] * w1s[None, None, :] + bc1, 0.0)
            h = np.maximum(h @ Wc2 + bc2, 0.0)
            a_hat[i0:i0 + 64] = 1.0 / (1.0 + np.exp(-(h @ Wc3 + bc3)))

    return a_hat, mu, logvar
